# revision 14
# baseline (speedup 1.0000x reference)
"""Trainium2 Bass kernel for nn_EquiformerWEdgesBackbone.

Strategy (8 NeuronCores, SPMD, one compiled program):
  - Edges are sharded by DESTINATION node range: core d owns nodes
    [750*d, 750*d+750) and all edges whose dst falls in that range.
    Per-destination softmax and the segment-sum scatter are then fully
    core-local; the only collective is an AllGather of the normalized
    node state (bf16) once per layer.
  - Node state is kept TRANSPOSED in SBUF: x_T [C=128 partitions, (k, n)]
    so that every matmul (attention tables, FFN, output proj) can use it
    directly as the stationary operand without any per-tile transposes.
  - Per layer, a gather table ytab [N, 1280] bf16 is built:
        cols 0:1152   y    = xn @ Wv      (k-major, (h,k,v) column order)
        cols 1152:1280 xn0 = xn[:, l=0, :]
    Edge tiles gather y rows (row gather) and xn0 rows (transposed
    gather) with one shared int16 index tile per 512-edge chunk.
  - Attention logits:  pre_T[c',e] = Wa1c.T@e_T + Wa1a.T@xn0_src_T
    + per-tile one-hot matmul for the dst term; silu on ACT; logits and
    gate via small matmuls; softmax WITHOUT max subtraction (logits are
    O(0.5), exp is safe); the division by the per-(node,head) sum is
    moved OUTSIDE the segment sum (it only depends on dst), so messages
    are scaled by w = exp(logits)*gate only.
  - Scatter: edges sorted by dst; per 128-node window the one-hot
    [edge,node] matmul accumulates agg[n, (h,k,v)] and the denominator
    sum (extra 8 columns) in PSUM across the window's edge tiles.

kernel(**inputs) takes the FULL inputs, preprocesses indices on host
(sort/pad/one-hot -- no model math), compiles once, runs on cores 0-7,
and reassembles the full [N, K, C] float32 output.
"""

import os
import sys
import numpy as np
import ml_dtypes

sys.path.insert(0, "/opt/trn_rl_repo")

import concourse.bass as bass
import concourse.mybir as mybir
import concourse.tile as tile
from concourse import bacc
from concourse.bass_utils import run_bass_kernel_spmd

F32 = mybir.dt.float32
BF16 = mybir.dt.bfloat16
I16 = mybir.dt.int16
AF = mybir.ActivationFunctionType

# ---------------- problem constants (hardcoded) ----------------
N = 6000
E = 150000
K = 9
C = 128
H = 8
V = 16
L = 2
B = 512
RMAX = 5.0
EPS = 1e-6
GW = RMAX / B           # gaussian width
NCORES = 8
NLOC = N // NCORES      # 750 owned nodes per core
NLOCP = 768             # padded to 6 windows of 128
NWIN = NLOCP // 128
KNP = K * NLOCP         # 6912  x_T free size
KNL = K * NLOC          # 6750  allgather payload per core
YW = K * C              # 1152 y columns, (h,k,v) order
TROW = YW + C           # 1280 table row


def _cdiv(a, b):
    return (a + b - 1) // b


# ============================================================
# program builder
# ============================================================

def build_program(TW, mock_cc=False, reps=1):
    """Build the SPMD Bass program. TW = tiles (of 128 edge slots) per
    128-node window, uniform across cores and windows, multiple of 4.
    mock_cc replaces the AllGather with local DMAs (for single-core
    cost-model profiling only -- wrong results on >1 core)."""
    NT = NWIN * TW          # edge tiles per core
    ECAP = NT * 128         # edge slots per core
    NCHUNK = NT // 4        # gather chunks (512 slots)

    nc = bacc.Bacc("TRN2", target_bir_lowering=False, debug=False,
                   num_devices=NCORES)

    # ---------------- DRAM I/O ----------------
    def din(name, shape, dt):
        return nc.dram_tensor(name, shape, dt, kind="ExternalInput")

    # weights / constants (identical on all cores)
    wrbf1_d = din("wrbf1", [512, C], BF16)
    w2_d = din("wrbf2", [C, C], BF16)
    b1_d = din("brbf1", [C, 1], F32)
    b2_d = din("brbf2", [C, 1], F32)
    cneg_d = din("cneg", [C, 4], F32)          # -centers/GW per basis tile
    atomtab_d = din("atomtab", [40, C], F32)
    bondtab_d = din("bondtab", [24, C], F32)
    wa1_d = din("wa1", [L, 3, C, C], BF16)     # [l, {a,b,c}, c, c']
    wa2_d = din("wa2", [L, C, H], BF16)
    wg_d = din("wg", [L, C, H], BF16)
    wv_d = din("wv", [L, C, C], BF16)          # columns in (h,v) order
    wo_d = din("wo", [L, C, C], BF16)          # rows in (h,v) order
    wf1_d = din("wf1", [L, C, 512], BF16)
    wf2_d = din("wf2", [L, 512, C], BF16)

    # per-core data
    aoh_d = din("aoh", [40, NLOCP], F32)       # atom one-hot (cols = own nodes)
    d_d = din("dist", [1, ECAP], F32)          # sorted/padded edge distances
    boh_d = din("boh", [24, ECAP], F32)        # bond one-hot (transposed)
    # one-hots packed window-major: [win, partition, tile*128] so a whole
    # window loads as 128 contiguous 7KB descriptors
    ohen_d = din("ohen", [NWIN, 128, TW * 128], BF16)  # [edge, node-in-window]
    ohne_d = din("ohne", [NWIN, 128, TW * 128], BF16)  # its transpose [node, edge]
    idx_d = din("idx", [128, NCHUNK * 32], I16)  # wrapped gather indices

    out_d = nc.dram_tensor("xout", [C, KNL], F32, kind="ExternalOutput")

    # internal DRAM
    ytab = nc.dram_tensor("ytab", [N, TROW], BF16, kind="Internal")
    agin = nc.dram_tensor("agin", [C, KNL], BF16, kind="Internal")
    agout = nc.dram_tensor("agout", [NCORES * C, KNL], BF16,
                           kind="Internal", addr_space="Shared")

    with tile.TileContext(nc) as tc:
        import contextlib
        ctx = contextlib.ExitStack()
        with ctx:
            wp = ctx.enter_context(tc.tile_pool(name="wp", bufs=1))
            big = ctx.enter_context(tc.tile_pool(name="big", bufs=1))

            # ---------- load weights to SBUF ----------
            def wtile(name, shape, dt, src_ap):
                t = wp.tile(shape, dt, tag=name)
                nc.sync.dma_start(out=t[:], in_=src_ap)
                return t

            w1t = wtile("w1t", [128, 4, C], BF16,
                        wrbf1_d.ap().rearrange("(t b) c -> b t c", b=128))
            w2t = wtile("w2t", [C, C], BF16, w2_d.ap())
            b1t = wtile("b1t", [C, 1], F32, b1_d.ap())
            b2t = wtile("b2t", [C, 1], F32, b2_d.ap())
            cnegt = wtile("cnegt", [C, 4], F32, cneg_d.ap())
            atomt = wtile("atomt", [40, C], F32, atomtab_d.ap())
            bondt = wtile("bondt", [24, C], F32, bondtab_d.ap())
            wa1t = wtile("wa1t", [C, L, 3, C], BF16,
                         wa1_d.ap().rearrange("l t c d -> c l t d"))
            wa2t = wtile("wa2t", [C, L, H], BF16,
                         wa2_d.ap().rearrange("l c h -> c l h"))
            wgt = wtile("wgt", [C, L, H], BF16,
                        wg_d.ap().rearrange("l c h -> c l h"))
            wvt = wtile("wvt", [C, L, C], BF16,
                        wv_d.ap().rearrange("l c d -> c l d"))
            wot = wtile("wot", [C, L, C], BF16,
                        wo_d.ap().rearrange("l c d -> c l d"))
            wf1t = wtile("wf1t", [C, L, 512], BF16,
                         wf1_d.ap().rearrange("l c f -> c l f"))
            wf2t = wtile("wf2t", [128, L, 4, C], BF16,
                         wf2_d.ap().rearrange("l (t b) c -> b l t c", b=128))

            identt = wp.tile([128, 128], BF16, tag="identt")
            from concourse.masks import make_identity
            make_identity(nc, identt[:])
            ones1 = wp.tile([1, 128], F32, tag="ones1")
            nc.vector.memset(ones1[:], 1.0)
            ones128 = wp.tile([128, 1], F32, tag="ones128")
            nc.vector.memset(ones128[:], 1.0)
            epst = wp.tile([1, 1], F32, tag="epst")
            nc.vector.memset(epst[:], float(EPS))

            # persistent SBUF state
            x_T = big.tile([C, KNP], F32, tag="x_T")
            e_T = big.tile([C, ECAP], BF16, tag="e_T")
            idxs = big.tile([128, NCHUNK * 32], I16, tag="idxs")
            nc.sync.dma_start(out=idxs[:], in_=idx_d.ap())
            xn_bf = big.tile([C, KNP], BF16, tag="xn_bf")
            t1w = big.tile([128, NWIN, C], BF16, tag="t1w")
            sig0 = big.tile([128, 4, NLOCP], BF16, tag="sig0")
            ssk = big.tile([1, NLOCP], F32, tag="ssk")
            rs = big.tile([1, NLOCP], F32, tag="rs")

            for _rep in range(reps):
                # ---------- P0: init x_T (atom embeddings into l=0) ----------
                with tc.tile_pool(name="p0ps", bufs=2, space="PSUM") as p0ps, \
                     tc.tile_pool(name="p0sb", bufs=1) as p0sb:
                    nc.vector.memset(x_T[:], 0.0)
                    aohs = p0sb.tile([40, NLOCP], F32, tag="aohs")
                    nc.sync.dma_start(out=aohs[:], in_=aoh_d.ap())
                    for j in range(2):  # two chunks of 384 node cols
                        ps = p0ps.tile([C, 384], F32, tag="a0")
                        nc.tensor.matmul(ps[:], lhsT=atomt[:],
                                         rhs=aohs[:, j * 384:(j + 1) * 384],
                                         start=True, stop=True)
                        nc.vector.tensor_copy(
                            out=x_T[:, j * 384:(j + 1) * 384], in_=ps[:])

                # ---------- helper: rms_sh normalize x_T -> dst ----------
                # squares go through a small per-k scratch (no [C, KNP]
                # f32 buffer); dst may alias x_T for the final in-place norm
                def rms_norm(dst_tile):
                    with tc.tile_pool(name="rmps", bufs=2, space="PSUM") as rmps, \
                         tc.tile_pool(name="rmsb", bufs=2) as rmsb:
                        # sum over c (partitions, via ones matmul) AND k
                        # (PSUM accumulation across the 9 k-blocks).
                        # NB: each matmul output must stay inside one PSUM
                        # bank (512 f32), so split 768 as 512+256.
                        ps = rmps.tile([1, 1024], F32, tag="ss")
                        for k in range(K):
                            sqk = rmsb.tile([C, NLOCP], F32, tag="sqk")
                            nc.scalar.activation(
                                out=sqk[:], in_=x_T[:, k * NLOCP:(k + 1) * NLOCP],
                                func=AF.Square)
                            for j0, j1 in ((0, 512), (512, NLOCP)):
                                nc.tensor.matmul(
                                    ps[:, j0:j1], lhsT=ones128[:],
                                    rhs=sqk[:, j0:j1],
                                    start=(k == 0), stop=(k == K - 1))
                        nc.vector.tensor_copy(out=ssk[:], in_=ps[:, 0:NLOCP])
                        nc.scalar.activation(out=rs[:], in_=ssk[:], func=AF.Sqrt,
                                             scale=1.0 / (K * C),
                                             bias=epst[:, 0:1])
                        nc.vector.reciprocal(out=rs[:], in_=rs[:])
                        rb = rmps.tile([C, 1024], F32, tag="rb")
                        nc.tensor.matmul(rb[:, 0:512], lhsT=ones1[:],
                                         rhs=rs[:, 0:512], start=True, stop=True)
                        nc.tensor.matmul(rb[:, 512:NLOCP], lhsT=ones1[:],
                                         rhs=rs[:, 512:NLOCP],
                                         start=True, stop=True)
                        nc.vector.tensor_tensor(
                            out=dst_tile[:].rearrange("c (k n) -> c k n", k=K),
                            in0=x_T[:].rearrange("c (k n) -> c k n", k=K),
                            in1=bass.AP(tensor=rb[:].tensor, offset=rb[:].offset,
                                        ap=[rb[:].ap[0], [0, K], [1, NLOCP]]),
                            op=mybir.AluOpType.mult)

                # ---------- per layer ----------
                for l in range(L):
                    rms_norm(xn_bf)

                    # allgather xn (valid nodes only)
                    nc.sync.dma_start(
                        out=agin.ap(),
                        in_=xn_bf[:].rearrange("c (k n) -> c k n", k=K)[:, :, :NLOC])
                    if mock_cc:
                        for dd in range(NCORES):
                            nc.sync.dma_start(
                                out=agout.ap()[dd * C:(dd + 1) * C, :],
                                in_=agin.ap())
                    else:
                        nc.gpsimd.collective_compute(
                            "AllGather", mybir.AluOpType.bypass,
                            replica_groups=[list(range(NCORES))],
                            ins=[agin.ap()], outs=[agout.ap()])

                    # table phase: t1w + ytab build; on layer 0 the edge
                    # feature pipeline (P1) is emitted in the same pool scope
                    # so its ACT/DVE work overlaps the AllGather + ytab DMA.
                    import contextlib as _ctl
                    _p1 = tc.tile_pool(name="p1ps", bufs=2, space="PSUM") \
                        if l == 0 else None
                    _p1e = tc.tile_pool(name="p1eps", bufs=1, space="PSUM") \
                        if l == 0 else None
                    _p1s = tc.tile_pool(name="p1sb", bufs=3) if l == 0 else None
                    with tc.tile_pool(name="t1ps", bufs=1, space="PSUM") as t1ps, \
                         tc.tile_pool(name="tbps", bufs=1, space="PSUM") as tbps, \
                         tc.tile_pool(name="tbxp", bufs=1, space="PSUM") as tbxp, \
                         tc.tile_pool(name="tbsb", bufs=2) as tbsb, \
                         (_p1 or _ctl.nullcontext()) as p1ps, \
                         (_p1e or _ctl.nullcontext()) as p1eps, \
                         (_p1s or _ctl.nullcontext()) as p1sb:
                        # t1w per window: [n, c'] = xn0_win.T @ Wa1b
                        for w in range(NWIN):
                            ps = t1ps.tile([128, C], F32, tag="t1")
                            nc.tensor.matmul(
                                ps[:], lhsT=xn_bf[:, w * 128:(w + 1) * 128],
                                rhs=wa1t[:, l, 1, :], start=True, stop=True)
                            nc.vector.tensor_copy(out=t1w[:, w, :], in_=ps[:])

                        if l == 0:
                            # ---- P1: edge features e_T [c, ECAP] ----
                            # (d-c)^2 on the DVE (idle here), Exp/Silu on ACT
                            # grouped so the table set switches 2x per group.
                            PG = 2
                            assert NCHUNK % PG == 0
                            for g0 in range(0, NCHUNK, PG):
                                h1s = []
                                for ch in range(g0, g0 + PG):
                                    dbc = p1sb.tile([128, 512], F32, tag="dbc")
                                    nc.sync.dma_start(
                                        out=dbc[:],
                                        in_=bass.AP(tensor=d_d, offset=ch * 512,
                                                    ap=[[0, 128], [1, 512]]))
                                    h1 = p1ps.tile([C, 512], F32, tag="h1")
                                    for bt in range(4):
                                        aff = p1sb.tile([128, 512], F32,
                                                        tag="aff")
                                        nc.vector.tensor_scalar(
                                            out=aff[:], in0=dbc[:],
                                            scalar1=1.0 / GW,
                                            scalar2=cnegt[:, bt:bt + 1],
                                            op0=mybir.AluOpType.mult,
                                            op1=mybir.AluOpType.add)
                                        sq = p1sb.tile([128, 512], BF16,
                                                       tag="sq")
                                        nc.vector.tensor_tensor(
                                            out=sq[:], in0=aff[:], in1=aff[:],
                                            op=mybir.AluOpType.mult)
                                        rbf = p1sb.tile([128, 512], BF16,
                                                        tag="rbf")
                                        nc.scalar.activation(out=rbf[:],
                                                             in_=sq[:],
                                                             func=AF.Exp,
                                                             scale=-1.0)
                                        nc.tensor.matmul(
                                            h1[:], lhsT=w1t[:, bt, :],
                                            rhs=rbf[:],
                                            start=(bt == 0), stop=(bt == 3))
                                    h1s.append(h1)
                                for i, ch in enumerate(range(g0, g0 + PG)):
                                    sl = slice(ch * 512, (ch + 1) * 512)
                                    hs = p1sb.tile([C, 512], BF16, tag="hs")
                                    nc.scalar.activation(out=hs[:],
                                                         in_=h1s[i][:],
                                                         func=AF.Silu,
                                                         bias=b1t[:, 0:1],
                                                         scale=1.0)
                                    ep = p1eps.tile([C, 512], F32, tag="ep")
                                    nc.tensor.matmul(ep[:], lhsT=w2t[:],
                                                     rhs=hs[:],
                                                     start=True, stop=False)
                                    bohc = p1sb.tile([24, 512], F32, tag="bohc")
                                    nc.sync.dma_start(out=bohc[:],
                                                      in_=boh_d.ap()[:, sl])
                                    nc.tensor.matmul(ep[:], lhsT=bondt[:],
                                                     rhs=bohc[:],
                                                     start=False, stop=True)
                                    nc.scalar.activation(out=e_T[:, sl],
                                                         in_=ep[:],
                                                         func=AF.Identity,
                                                         bias=b2t[:, 0:1],
                                                         scale=1.0)

                        # ---- build gather table ytab = [y | xn0] ----
                        # rows stay in natural k-major (k, h, v) order: the
                        # PSUM tile is copied out contiguously (ACT Copy)
                        jsz = [128] * 5 + [NLOC - 640]
                        for dcore in range(NCORES):
                            xng = tbsb.tile([C, KNL], BF16, tag="xng")
                            nc.sync.dma_start(
                                out=xng[:],
                                in_=agout.ap()[dcore * C:(dcore + 1) * C, :])
                            for j in range(6):
                                cn = jsz[j]
                                j0 = j * 128
                                ysb = tbsb.tile([128, TROW], BF16, tag="ysb")
                                yp = tbps.tile([128, YW], F32, tag="yp")
                                for k in range(K):
                                    nc.tensor.matmul(
                                        yp[:cn, k * C:(k + 1) * C],
                                        lhsT=xng[:, k * NLOC + j0:k * NLOC + j0 + cn],
                                        rhs=wvt[:, l, :], start=True, stop=True)
                                nc.scalar.activation(out=ysb[:cn, 0:YW],
                                                     in_=yp[:cn, :],
                                                     func=AF.Copy)
                                tp = tbxp.tile([128, C], BF16, tag="x0t")
                                nc.tensor.transpose(
                                    out=tp[:cn, :], in_=xng[:, j0:j0 + cn],
                                    identity=identt[:])
                                nc.vector.tensor_copy(out=ysb[:cn, YW:TROW],
                                                      in_=tp[:cn, :])
                                r0 = dcore * NLOC + j0
                                nc.sync.dma_start(
                                    out=ytab.ap()[r0:r0 + cn, :],
                                    in_=ysb[:cn, :])

                    # ---------- edge loop ----------
                    # Window-level phasing keeps the ACT engine on one
                    # function table at a time: per window, all pre tiles are
                    # copied out (Copy, in every table set), then one Silu,
                    # one Exp and one Sigmoid over the whole window's
                    # logits/gates instead of per-tile activations.
                    TWC = TW // 4
                    KV = K * V
                    with tc.tile_pool(name="agps", bufs=1, space="PSUM") as agps, \
                         tc.tile_pool(name="sps", bufs=1, space="PSUM") as sps, \
                         tc.tile_pool(name="pps", bufs=1, space="PSUM") as pps, \
                         tc.tile_pool(name="lps", bufs=1, space="PSUM") as lps, \
                         tc.tile_pool(name="mps", bufs=1, space="PSUM") as mps, \
                         tc.tile_pool(name="esb", bufs=3) as esb, \
                         tc.tile_pool(name="wsb", bufs=2) as wsb, \
                     tc.tile_pool(name="episb", bufs=1) as episb, \
                         tc.tile_pool(name="gsb", bufs=2) as gsb:
                        for w in range(NWIN):
                            agg = agps.tile([128, YW], F32, tag="agg")
                            sden = sps.tile([128, H], F32, tag="sden")
                            preR = wsb.tile([C, TWC, 512], BF16, tag="preR")
                            # whole window's one-hots in two DMAs
                            ohneW = wsb.tile([128, TW, 128], BF16, tag="ohneW")
                            nc.sync.dma_start(
                                out=ohneW[:],
                                in_=ohne_d.ap()[w].rearrange(
                                    "p (t e) -> p t e", t=TW))
                            ohenW = wsb.tile([128, TW, 128], BF16, tag="ohenW")
                            nc.sync.dma_start(
                                out=ohenW[:],
                                in_=ohen_d.ap()[w].rearrange(
                                    "p (t e) -> p t e", t=TW))
                            # ---- phase A: pre activations per chunk ----
                            for chi in range(TWC):
                                ch = w * TWC + chi
                                x0b = gsb.tile([128, 1, 512], BF16, tag="x0b")
                                nc.gpsimd.dma_gather(
                                    x0b[:], ytab.ap()[:, YW:TROW],
                                    idxs[:, ch * 32:(ch + 1) * 32],
                                    512, 512, C, elem_step=TROW,
                                    transpose=True)
                                pre = pps.tile([C, 512], F32, tag="pre")
                                nc.tensor.matmul(
                                    pre[:], lhsT=wa1t[:, l, 2, :],
                                    rhs=e_T[:, ch * 512:(ch + 1) * 512],
                                    start=True, stop=False)
                                nc.tensor.matmul(
                                    pre[:], lhsT=wa1t[:, l, 0, :],
                                    rhs=x0b[:, 0, :], start=False, stop=False)
                                for sub in range(4):
                                    ti = chi * 4 + sub
                                    nc.tensor.matmul(
                                        pre[:, sub * 128:(sub + 1) * 128],
                                        lhsT=t1w[:, w, :],
                                        rhs=ohneW[:, ti, :],
                                        start=False, stop=(sub == 3))
                                nc.scalar.activation(out=preR[:, chi, :],
                                                     in_=pre[:], func=AF.Copy)
                            # ---- phase B: batched silu/exp/sigmoid ----
                            preS = wsb.tile([C, TWC, 512], BF16, tag="preS")
                            nc.scalar.activation(out=preS[:], in_=preR[:],
                                                 func=AF.Silu)
                            lgW = lps.tile([128, TW, 2 * H], F32, tag="lgW")
                            for ti in range(TW):
                                tg = w * TW + ti
                                nc.tensor.matmul(
                                    lgW[:, ti, 0:H],
                                    lhsT=preS[:, ti // 4,
                                              (ti % 4) * 128:(ti % 4 + 1) * 128],
                                    rhs=wa2t[:, l, :], start=True, stop=True)
                                nc.tensor.matmul(
                                    lgW[:, ti, H:2 * H],
                                    lhsT=e_T[:, tg * 128:(tg + 1) * 128],
                                    rhs=wgt[:, l, :], start=True, stop=True)
                            exbW = wsb.tile([128, TW, H], BF16, tag="exbW")
                            nc.scalar.activation(out=exbW[:],
                                                 in_=lgW[:, :, 0:H],
                                                 func=AF.Exp)
                            gtW = wsb.tile([128, TW, H], BF16, tag="gtW")
                            nc.scalar.activation(out=gtW[:],
                                                 in_=lgW[:, :, H:2 * H],
                                                 func=AF.Sigmoid)
                            wbW = wsb.tile([128, TW, H], BF16, tag="wbW")
                            nc.vector.tensor_tensor(
                                out=wbW[:], in0=exbW[:], in1=gtW[:],
                                op=mybir.AluOpType.mult)
                            # ---- phase C: gather / scale / scatter ----
                            for chi in range(TWC):
                                ch = w * TWC + chi
                                ybuf = gsb.tile([128, 4, YW], BF16, tag="ybuf")
                                nc.gpsimd.dma_gather(
                                    ybuf[:], ytab.ap()[:, 0:YW],
                                    idxs[:, ch * 32:(ch + 1) * 32],
                                    512, 512, YW, elem_step=TROW)
                                for sub in range(4):
                                    ti = chi * 4 + sub
                                    # ybuf rows are k-major (k, h, v); the
                                    # per-(edge, head) weight broadcasts over
                                    # k and v via a rank-4 stride-0 AP
                                    msk = esb.tile([128, YW], BF16, tag="msk")
                                    nc.vector.tensor_tensor(
                                        out=msk[:].rearrange(
                                            "e (k h v) -> e k h v", k=K, h=H),
                                        in0=ybuf[:, sub, :].rearrange(
                                            "e (k h v) -> e k h v", k=K, h=H),
                                        in1=bass.AP(
                                            tensor=wbW[:].tensor,
                                            offset=wbW[:].offset + ti * H,
                                            ap=[wbW[:].ap[0], [0, K], [1, H],
                                                [0, V]]),
                                        op=mybir.AluOpType.mult)
                                    st = (chi == 0 and sub == 0)
                                    sp = (chi == TWC - 1 and sub == 3)
                                    nc.tensor.matmul(agg[:, 0:512],
                                                     lhsT=ohenW[:, ti, :],
                                                     rhs=msk[:, 0:512],
                                                     start=st, stop=sp)
                                    nc.tensor.matmul(agg[:, 512:1024],
                                                     lhsT=ohenW[:, ti, :],
                                                     rhs=msk[:, 512:1024],
                                                     start=st, stop=sp)
                                    nc.tensor.matmul(agg[:, 1024:YW],
                                                     lhsT=ohenW[:, ti, :],
                                                     rhs=msk[:, 1024:YW],
                                                     start=st, stop=sp)
                                    nc.tensor.matmul(sden[:],
                                                     lhsT=ohenW[:, ti, :],
                                                     rhs=exbW[:, ti, :],
                                                     start=st, stop=sp)
                            # ----- window epilogue -----
                            rcp = episb.tile([128, H], F32, tag="rcp")
                            nc.vector.tensor_scalar_add(out=rcp[:],
                                                        in0=sden[:],
                                                        scalar1=1e-9)
                            nc.vector.reciprocal(out=rcp[:], in_=rcp[:])
                            # normalized agg is already k-major: feed the
                            # per-k transposes directly, no interleave copy
                            aggn = episb.tile([128, YW], BF16, tag="aggn")
                            nc.vector.tensor_tensor(
                                out=aggn[:].rearrange(
                                    "n (k h v) -> n k h v", k=K, h=H),
                                in0=agg[:].rearrange(
                                    "n (k h v) -> n k h v", k=K, h=H),
                                in1=bass.AP(
                                    tensor=rcp[:].tensor,
                                    offset=rcp[:].offset,
                                    ap=[rcp[:].ap[0], [0, K], [1, H], [0, V]]),
                                op=mybir.AluOpType.mult)
                            for k in range(K):
                                tp = mps.tile([128, 128], BF16, tag="atp")
                                nc.tensor.transpose(
                                    out=tp[:],
                                    in_=aggn[:, k * 128:(k + 1) * 128],
                                    identity=identt[:])
                                aT = esb.tile([128, 128], BF16, tag="aT")
                                nc.vector.tensor_copy(out=aT[:], in_=tp[:])
                                dk = mps.tile([128, 128], F32, tag="dk")
                                nc.tensor.matmul(dk[:], lhsT=wot[:, l, :],
                                                 rhs=aT[:], start=True, stop=True)
                                nc.vector.tensor_tensor(
                                    out=x_T[:, k * NLOCP + w * 128:
                                            k * NLOCP + (w + 1) * 128],
                                    in0=x_T[:, k * NLOCP + w * 128:
                                            k * NLOCP + (w + 1) * 128],
                                    in1=dk[:], op=mybir.AluOpType.add)

                    # ---------- FFN ----------
                    rms_norm(xn_bf)
                    with tc.tile_pool(name="fps", bufs=2, space="PSUM") as fps, \
                         tc.tile_pool(name="fsb", bufs=3) as fsb:
                        # col chunks of 384: 18 chunks; chunks 0,1 are k=0
                        for j in range(18):
                            c0 = j * 384
                            dlt = fps.tile([C, 384], F32, tag="dlt")
                            for fc in range(4):
                                hp = fps.tile([128, 384], F32, tag="hp")
                                nc.tensor.matmul(
                                    hp[:], lhsT=wf1t[:, l, fc * 128:(fc + 1) * 128],
                                    rhs=xn_bf[:, c0:c0 + 384],
                                    start=True, stop=True)
                                hb = fsb.tile([128, 384], BF16, tag="hb")
                                if j < 2:
                                    # silu(x) = x * sigmoid(x): reuse the
                                    # stored sigmoid, no Silu table load
                                    nc.scalar.activation(
                                        out=sig0[:, fc, j * 384:(j + 1) * 384],
                                        in_=hp[:], func=AF.Sigmoid)
                                nsl = slice((j % 2) * 384, (j % 2) * 384 + 384)
                                nc.vector.tensor_tensor(
                                    out=hb[:], in0=hp[:],
                                    in1=sig0[:, fc, nsl],
                                    op=mybir.AluOpType.mult)
                                nc.tensor.matmul(
                                    dlt[:], lhsT=wf2t[:, l, fc, :], rhs=hb[:],
                                    start=(fc == 0), stop=(fc == 3))
                            nc.vector.tensor_tensor(
                                out=x_T[:, c0:c0 + 384],
                                in0=x_T[:, c0:c0 + 384],
                                in1=dlt[:], op=mybir.AluOpType.add)

                # ---------- final norm (in place) + output ----------
                rms_norm(x_T)
                nc.sync.dma_start(
                    out=out_d.ap(),
                    in_=x_T[:].rearrange("c (k n) -> c k n", k=K)[:, :, :NLOC])

    nc.compile()
    return nc


# ============================================================
# host preprocessing + runner
# ============================================================

_CACHE = {}


def _prep(inputs):
    """Index-only host preprocessing; returns (TW, per-core in_maps)."""
    atom_feats = np.asarray(inputs["atom_feats"]).astype(np.int64)
    bond_feats = np.asarray(inputs["bond_feats"]).astype(np.int64)
    edge_index = np.asarray(inputs["edge_index"]).astype(np.int64)
    edge_distance = np.asarray(inputs["edge_distance"]).astype(np.float32)

    src, dst = edge_index[0], edge_index[1]

    # ---- per-core edge partition by dst range, sorted by dst ----
    cores = []
    maxcnt = 0
    for d in range(NCORES):
        sel = np.nonzero((dst >= d * NLOC) & (dst < (d + 1) * NLOC))[0]
        dl = (dst[sel] - d * NLOC).astype(np.int64)
        order = np.argsort(dl, kind="stable")
        sel = sel[order]
        dl = dl[order]
        wins = dl // 128
        cnts = np.bincount(wins, minlength=NWIN)
        maxcnt = max(maxcnt, int(cnts.max()))
        cores.append((sel, dl, cnts))
    TW = _cdiv(maxcnt, 128)
    TW = _cdiv(TW, 4) * 4
    NT = NWIN * TW
    ECAP = NT * 128
    NCHUNK = NT // 4

    # ---- weights ----
    f32 = np.float32
    bf16 = ml_dtypes.bfloat16
    W_rbf1 = np.asarray(inputs["W_rbf1"], f32)
    W_rbf2 = np.asarray(inputs["W_rbf2"], f32)
    b_rbf1 = np.asarray(inputs["b_rbf1"], f32)
    b_rbf2 = np.asarray(inputs["b_rbf2"], f32)
    Wa1 = np.asarray(inputs["Wa1"], f32)
    Wa2 = np.asarray(inputs["Wa2"], f32)
    Wv = np.asarray(inputs["Wv"], f32)
    Wg = np.asarray(inputs["Wg"], f32)
    Wo = np.asarray(inputs["Wo"], f32)
    Wf1 = np.asarray(inputs["Wf1"], f32)
    Wf2 = np.asarray(inputs["Wf2"], f32)
    atom_emb = np.asarray(inputs["atom_emb"], f32)
    bond_emb = np.asarray(inputs["bond_emb"], f32)

    centers = np.linspace(0.0, RMAX, B).astype(f32)
    cneg = (-centers / GW).reshape(4, 128).T.copy()       # [128, 4]

    common = {
        "wrbf1": W_rbf1.astype(bf16),
        "wrbf2": W_rbf2.astype(bf16),
        "brbf1": b_rbf1.reshape(C, 1),
        "brbf2": b_rbf2.reshape(C, 1),
        "cneg": np.ascontiguousarray(cneg),
        "atomtab": atom_emb.reshape(40, C).copy(),
        "bondtab": bond_emb.reshape(24, C).copy(),
        "wa1": np.ascontiguousarray(
            Wa1.reshape(L, 3, C, C)).astype(bf16),
        "wa2": Wa2.astype(bf16),
        "wg": Wg.astype(bf16),
        "wv": Wv.astype(bf16),
        "wo": Wo.astype(bf16),
        "wf1": Wf1.astype(bf16),
        "wf2": Wf2.astype(bf16),
    }

    in_maps = []
    for d in range(NCORES):
        sel, dl, cnts = cores[d]
        # slot layout: window w occupies tiles [w*TW, (w+1)*TW)
        slot_src = np.zeros(ECAP, np.int64)
        slot_dln = np.zeros(ECAP, np.int64)   # dst-in-window
        slot_valid = np.zeros(ECAP, bool)
        pos = 0
        for w in range(NWIN):
            cnt = int(cnts[w])
            base = w * TW * 128
            slot_src[base:base + cnt] = src[sel[pos:pos + cnt]]
            slot_dln[base:base + cnt] = dl[pos:pos + cnt] - w * 128
            slot_valid[base:base + cnt] = True
            pos += cnt

        # one-hot [edge, node] per tile (+ transpose)
        ohen = np.zeros((NT, 128, 128), bf16)
        tl = np.arange(ECAP)
        tn, te = tl // 128, tl % 128
        v = slot_valid
        ohen[tn[v], te[v], slot_dln[v]] = 1.0
        ohne = np.ascontiguousarray(ohen.transpose(0, 2, 1))
        # pack window-major: [win, partition, tile*128]
        ohen = np.ascontiguousarray(
            ohen.reshape(NWIN, TW, 128, 128).transpose(0, 2, 1, 3)
            .reshape(NWIN, 128, TW * 128))
        ohne = np.ascontiguousarray(
            ohne.reshape(NWIN, TW, 128, 128).transpose(0, 2, 1, 3)
            .reshape(NWIN, 128, TW * 128))

        # distances / bond one-hot
        dist = np.zeros(ECAP, f32)
        dist[slot_valid] = edge_distance[sel]
        boh = np.zeros((24, ECAP), f32)
        for f in range(3):
            boh[f * 8 + bond_feats[sel, f], np.nonzero(slot_valid)[0]] = 1.0

        # atom one-hot (own nodes, padded cols)
        aoh = np.zeros((40, NLOCP), f32)
        own = np.arange(d * NLOC, (d + 1) * NLOC)
        for f in range(4):
            aoh[f * 10 + atom_feats[own, f], np.arange(NLOC)] = 1.0

        # wrapped int16 gather indices
        idx = np.zeros((16, NCHUNK * 32), np.int16)
        g = np.arange(ECAP)
        ci, ii = g // 512, g % 512
        idx[ii % 16, ci * 32 + ii // 16] = slot_src.astype(np.int16)
        idx = np.tile(idx, (8, 1))

        m = dict(common)
        m.update({
            "aoh": aoh, "dist": dist.reshape(1, ECAP), "boh": boh,
            "ohen": ohen, "ohne": ohne, "idx": idx,
        })
        in_maps.append(m)
    return TW, in_maps


def _get_nc(TW, reps=1):
    key = ('nc', TW, reps)
    if key not in _CACHE:
        _CACHE[key] = build_program(TW, reps=reps)
    return _CACHE[key]


def _make_runner(nc, reps):
    """jit-compiled SPMD runner that chains the NEFF `reps` times
    back-to-back (outputs fed back as the donated output operands), so
    (T(reps) - T(1)) / (reps - 1) cancels host/axon dispatch overhead."""
    import jax
    from jax.sharding import Mesh, PartitionSpec
    from jax.experimental.shard_map import shard_map
    from concourse import bass2jax
    import concourse.mybir as mb

    bass2jax.install_neuronx_cc_hook()
    part_name = (nc.partition_id_tensor.name
                 if nc.partition_id_tensor else None)
    in_names, out_names, out_avals, zero_outs = [], [], [], []
    for alloc in nc.m.functions[0].allocations:
        if not isinstance(alloc, mybir.MemoryLocationSet):
            continue
        name = alloc.memorylocations[0].name
        if alloc.kind == "ExternalInput":
            if name != part_name:
                in_names.append(name)
        elif alloc.kind == "ExternalOutput":
            out_names.append(name)
            shape = tuple(alloc.tensor_shape)
            dtype = mb.dt.np(alloc.dtype)
            out_avals.append(jax.core.ShapedArray(shape, dtype))
            zero_outs.append(np.zeros(shape, dtype))
    n_params = len(in_names)
    all_names = list(in_names) + list(out_names)
    if part_name is not None:
        all_names.append(part_name)

    def _body(*args):
        o = list(args[n_params:])
        for _ in range(reps):
            ops = list(args[:n_params]) + o
            if part_name is not None:
                ops.append(bass2jax.partition_id_tensor())
            o = list(bass2jax._bass_exec_p.bind(
                *ops,
                out_avals=tuple(out_avals),
                in_names=tuple(all_names),
                out_names=tuple(out_names),
                lowering_input_output_aliases=(),
                sim_require_finite=True,
                sim_require_nnan=True,
                nc=nc))
        return tuple(o)

    devices = jax.devices()[:NCORES]
    mesh = Mesh(np.asarray(devices), ("core",))
    n_outs = len(out_names)
    in_specs = (PartitionSpec("core"),) * (n_params + n_outs)
    out_specs = (PartitionSpec("core"),) * n_outs
    donate = tuple(range(n_params, n_params + n_outs))
    fn = jax.jit(
        shard_map(_body, mesh=mesh, in_specs=in_specs,
                  out_specs=out_specs, check_rep=False),
        donate_argnums=donate, keep_unused=True)
    return fn, mesh, in_names, out_names, out_avals, zero_outs


def _concat_inputs(in_maps, in_names):
    return [np.concatenate([np.asarray(in_maps[c][n]) for c in range(NCORES)],
                           axis=0) for n in in_names]


def _unpack_out(arrs, out_avals):
    # arrs[i]: [NCORES*dim0, ...] -> full [N,K,C]
    xo = np.asarray(arrs[0]).reshape(NCORES, C, K, NLOC)
    out = np.zeros((N, K, C), np.float32)
    for d in range(NCORES):
        out[d * NLOC:(d + 1) * NLOC] = xo[d].transpose(2, 1, 0)
    return out


def _run(TW, in_maps, reps_timing=0):
    """Returns (out, timing_info)."""
    import jax, time
    nc = _get_nc(TW)
    key = (TW, 1)
    if key not in _CACHE:
        _CACHE[key] = _make_runner(nc, 1)
    fn1, mesh, in_names, out_names, out_avals, zero_outs = _CACHE[key]
    cin = _concat_inputs(in_maps, in_names)
    czo = [np.zeros((NCORES * z.shape[0], *z.shape[1:]), z.dtype)
           for z in zero_outs]
    outs = fn1(*cin, *czo)
    jax.block_until_ready(outs)
    result = _unpack_out(outs, out_avals)

    timing = None
    if reps_timing:
        from jax.sharding import NamedSharding, PartitionSpec
        shard = NamedSharding(mesh, PartitionSpec("core"))
        cin_dev = [jax.device_put(a, shard) for a in cin]

        def seq_times(f, n):
            ts = []
            for _ in range(n):
                z = [jax.device_put(np.zeros_like(a), shard) for a in czo]
                jax.block_until_ready(z)
                t0 = time.perf_counter()
                o = f(*cin_dev, *z)
                jax.block_until_ready(o)
                ts.append(time.perf_counter() - t0)
            return ts

        seq_times(fn1, 2)  # warm
        ts1 = seq_times(fn1, 8)
        R = reps_timing
        tsR = None
        if R > 1:
            ncR = _get_nc(TW, reps=R)
            kr = ("fn", TW, R)
            if kr not in _CACHE:
                _CACHE[kr] = _make_runner(ncR, 1)
            fnR = _CACHE[kr][0]
            seq_times(fnR, 1)  # warm/compile
            tsR = seq_times(fnR, 8)
            per_iter = (min(tsR) - min(ts1)) / (R - 1)
        else:
            per_iter = min(ts1)
        timing = dict(ts1=ts1, tsR=tsR, reps=R, per_iter=per_iter)
    return result, timing


_NOOP = {}


def _noop_floor(n):
    """Min wall-clock of a trivial 8-core bass program = dispatch floor."""
    import jax, time
    if "fn" not in _NOOP:
        nnc = bacc.Bacc("TRN2", target_bir_lowering=False, debug=False,
                        num_devices=NCORES)
        a_d = nnc.dram_tensor("a", [128, 128], F32, kind="ExternalInput")
        b_d = nnc.dram_tensor("b", [128, 128], F32, kind="ExternalOutput")
        with tile.TileContext(nnc) as ntc:
            with ntc.tile_pool(name="p", bufs=1) as p:
                t = p.tile([128, 128], F32)
                nnc.sync.dma_start(out=t[:], in_=a_d.ap())
                nnc.sync.dma_start(out=b_d.ap(), in_=t[:])
        nnc.compile()
        _NOOP["fn"] = _make_runner(nnc, 1)
    fn, mesh, in_names, out_names, out_avals, zero_outs = _NOOP["fn"]
    ain = np.zeros((NCORES * 128, 128), np.float32)
    ain_dev = jax.device_put(ain)
    best = float("inf")
    fn(ain_dev, jax.device_put(np.zeros_like(ain)))
    for _ in range(n):
        z = jax.device_put(np.zeros_like(ain))
        jax.block_until_ready(z)
        t0 = time.perf_counter()
        o = fn(ain_dev, z)
        jax.block_until_ready(o)
        best = min(best, time.perf_counter() - t0)
    return best


def kernel(**inputs):
    TW, in_maps = _prep(inputs)
    out, _ = _run(TW, in_maps)
    return out

